# revision 28
# baseline (speedup 1.0000x reference)
"""ConvAttention Trainium2 kernel (Bass/Tile), data-parallel over batch on 8
NeuronCores (1 batch per core, weights broadcast).

Reference computation (per batch b):
  q = conv3d(input, wq, 1x3x3, pad (0,1,1)) + bq, scaled by 0.5
  k = conv3d(memory, wk, 1x3x3, pad (0,1,1)) + bk
  v = conv3d(memory, wv, 3x3x3, pad (0,1,1)) + bv        (depth valid: L-2)
  heads split depth: q,k -> (2, 128, 9*32*32), v -> (2, 128, 8*32*32)
  logit[h] = q[h] @ k[h].T -> softmax over last axis -> @ v[h]
  out (128, 16, 32, 32)

Kernel design per core (v4 — gap hunting on top of v3's packing):
  - Every SBUF image tile is ONE HBM DMA from a host-side packed layout:
      L1 tile = [x | x<<1]        (partition halves; shift = 1 flat column)
      L2 tile = [x<<2 | x<<36]    (col-shift 2 | row-shift 1 + col-shift 2)
      D tile  = mem slice j unshifted (single, [128, SLICE] even|odd halves)
  - q/k 1x3x3 convs: per 16-row output tile, 5 matmuls: 3 K=128 L1-pairs
    (taps (dy,0)+(dy,1)), 1 K=128 L2-pair ((0,2)+(1,2)), 1 K=64 single
    ((2,2)).  20 matmuls/slice.
  - v 3x3x3 conv: output lv pairs two of its three input slices in a D tile
    (same tap => same window offset for both halves => all 9 spatial taps are
    K=128), the third (solo) slice uses the L1/L2 packing: per tile
    9 + 3 + 1 + 1 = 14 matmuls.  28 matmuls/slice.
    Even lv: pair (lv,lv+1) + solo lv+2; odd lv: pair (lv+1,lv+2) + solo lv.
  - Dtype split: fp32r matmuls self-load weights (no exposed ~60ns LDWEIGHTS)
    but cost 2x DMA bytes; per-core HBM (~300-360 GB/s) binds on bursts. So
    the matmul-heavy L1 + D paths run fp32r and the small L2 path runs fp16.
  - v4 scheduling fixes (PE has a p-state ramp: every idle gap also costs up
    to ~3us of half-clock ramp after it, so gaps are the enemy):
      * md prefetched as 9 spread-out singles on the sync queue (was 1.2MB
        pair bursts on SWDGE that delayed slice loads and starved the PE).
      * logit deferral depth 3 (was 2): qkT transposes get a full extra slice
        of slack before the fp16 logit matmuls need them.
      * head-0 epilogue shifted to l=11..14; head-1 tail flushes the two
        ready pending slices BEFORE the last v tile so only slice 17's own
        logits wait on its transpose.
      * attn psum->sbuf copies alternate DVE / ACT (gpsimd has no PSUM port;
        single-engine copies at ~650ns were the tail PE bottleneck vs 213ns
        matmuls).
      * output stores on the ACT HWDGE queue (SWDGE pays ~1us fixed cost per
        dma_start and the sync queue must stay pure prefetch).
      * slices 0-1 load in two column halves so the first conv matmuls start
        after ~55% of the bytes (shrinks the per-rep pipeline fill; the
        timing loop has an all-engine barrier per rep so fill counts).
"""
import numpy as np

import concourse.bacc as bacc
import concourse.mybir as mybir
import concourse.tile as tile
from concourse import bass_utils

F32 = mybir.dt.float32
F32R = mybir.dt.float32r
F16 = mybir.dt.float16

B, CIN, COUT, L, H, W = 8, 64, 128, 18, 32, 32
NH = 2              # heads
DQ = L // NH        # 9 depth slices per head for q/k
LV = L - 2          # 16 v depth slices
DV = LV // NH       # 8 per head
HP, WP = H + 2, W + 2          # padded spatial
SLICE = HP * WP                # 1156
NPOS = H * W                   # 1024 positions per depth slice
DEPTH_SCALE = 0.5

# ---- variant knobs ----
QK_DT = F32R        # L1 conv path (q/k pairs + v solo L1)
L2_DT = F16         # L2 conv path (q/k dx=2 taps + v solo dx=2)
VD_DT = F32R        # v D-pair path
HILO = False        # evict q as hi+lo fp16 and use 2-term logits

# kept for test.py compatibility (unused)
QK_F32R = True
SPLIT_LOGITS = False

_CACHE = {}

# logit deferral depth: qkT of slice l is consumed at slice l+DEFER.
# The transposes are paced ~1.6 slices behind the PE by the per-lane DMA
# FIFO bookkeeping, so the emits need real margin on top of that.
DEFER = 4
# qkT buffers: DEFER+2 so the transpose of slice l only waits for the logit
# emit of slice l-5 (one extra slice of slack vs the minimum DEFER+1 — the
# transposes pace the serial ACT queue, so slack here decouples everything
# downstream of them)
QKT_BUFS = DEFER + 2
# d_j (md single-slice) prefetch schedule: {iteration l: j}
D_SCHED = {1: 0, 2: 1, 3: 2, 5: 3, 7: 4, 9: 5, 11: 6, 13: 7, 15: 8}
# column split for the slice-0/1 two-part loads (tile 0 reads cols < 646)
CSPL = 19 * WP


def _np(dt):
    return np.float16 if dt == F16 else np.float32


def build_module(reps=1, **_ignored):
    """reps>1 wraps the whole computation in a hardware loop — used only for
    timing (amortizes the per-dispatch overhead of the execution path)."""
    nc = bacc.Bacc("TRN2", target_bir_lowering=False, debug=False)

    xa1 = nc.dram_tensor("xa1", [128, L, SLICE], QK_DT, kind="ExternalInput").ap()
    xa2 = nc.dram_tensor("xa2", [128, L, SLICE], L2_DT, kind="ExternalInput").ap()
    ma1 = nc.dram_tensor("ma1", [128, L, SLICE], QK_DT, kind="ExternalInput").ap()
    ma2 = nc.dram_tensor("ma2", [128, L, SLICE], L2_DT, kind="ExternalInput").ap()
    md = nc.dram_tensor("md", [128, 9, SLICE], VD_DT, kind="ExternalInput").ap()
    # weights: L1 pairs [128, 3, 128] (rows 0:64 tap (dy,0), 64:128 tap (dy,1))
    wqp = nc.dram_tensor("wqp", [128, 3, 128], QK_DT, kind="ExternalInput").ap()
    wkp = nc.dram_tensor("wkp", [128, 3, 128], QK_DT, kind="ExternalInput").ap()
    # L2 pairs [:,0]=q ((0,2)|(1,2)), [:,1]=k; singles (2,2) dup'd both halves
    wqk2 = nc.dram_tensor("wqk2", [128, 2, 128], L2_DT, kind="ExternalInput").ap()
    wqks = nc.dram_tensor("wqks", [128, 2, 128], L2_DT, kind="ExternalInput").ap()
    # v weights: D-pairs [:, par, i(dy,dx)]; par0 = (dl0|dl1), par1 = (dl1|dl2)
    wvd = nc.dram_tensor("wvd", [128, 2, 9, 128], VD_DT, kind="ExternalInput").ap()
    # v solo: par0 -> dl2, par1 -> dl0
    wvl1 = nc.dram_tensor("wvl1", [128, 2, 3, 128], QK_DT, kind="ExternalInput").ap()
    wvl2 = nc.dram_tensor("wvl2", [128, 2, 128], L2_DT, kind="ExternalInput").ap()
    wvs = nc.dram_tensor("wvs", [128, 2, 128], L2_DT, kind="ExternalInput").ap()
    bq = nc.dram_tensor("bq", [128, 1], F32, kind="ExternalInput").ap()
    bk = nc.dram_tensor("bk", [128, 1], F32, kind="ExternalInput").ap()
    bv = nc.dram_tensor("bv", [128, 1], F32, kind="ExternalInput").ap()
    ident = nc.dram_tensor("ident", [128, 128], F16, kind="ExternalInput").ap()
    out = nc.dram_tensor("out", [128, LV * NPOS], F16, kind="ExternalOutput").ap()

    QW = 2048          # width of the fused [q|k] tile
    KOFF = 1024        # column offset of the k block

    with tile.TileContext(nc) as tc:
        with tc.tile_pool(name="consts", bufs=1) as cpool, \
             tc.tile_pool(name="xin1", bufs=4) as xin1_pool, \
             tc.tile_pool(name="xin2", bufs=4) as xin2_pool, \
             tc.tile_pool(name="xm1", bufs=6) as xm1_pool, \
             tc.tile_pool(name="xm2", bufs=6) as xm2_pool, \
             tc.tile_pool(name="xd", bufs=3) as xd_pool, \
             tc.tile_pool(name="qkc", bufs=3) as qkc_pool, \
             tc.tile_pool(name="qkT", bufs=QKT_BUFS) as qkT_pool, \
             tc.tile_pool(name="vall", bufs=1) as vall_pool, \
             tc.tile_pool(name="sm", bufs=2) as sm_pool, \
             tc.tile_pool(name="ost", bufs=3) as ost_pool, \
             tc.tile_pool(name="pconv", bufs=5, space="PSUM") as pconv, \
             tc.tile_pool(name="pattn", bufs=1, space="PSUM") as pattn, \
             tc.tile_pool(name="plogit", bufs=2, space="PSUM") as plogit:

            wqp_t = cpool.tile([128, 3, 128], QK_DT)
            wkp_t = cpool.tile([128, 3, 128], QK_DT)
            wqk2_t = cpool.tile([128, 2, 128], L2_DT)
            wqks_t = cpool.tile([128, 2, 128], L2_DT)
            wvd_t = cpool.tile([128, 2, 9, 128], VD_DT)
            wvl1_t = cpool.tile([128, 2, 3, 128], QK_DT)
            wvl2_t = cpool.tile([128, 2, 128], L2_DT)
            wvs_t = cpool.tile([128, 2, 128], L2_DT)
            bq_t = cpool.tile([128, 1], F32)
            bk_t = cpool.tile([128, 1], F32)
            bv_t = cpool.tile([128, 1], F32)
            ident_t = cpool.tile([128, 128], F16)
            for t, d in [(wqp_t, wqp), (wkp_t, wkp), (wqk2_t, wqk2),
                         (wqks_t, wqks), (bq_t, bq), (bk_t, bk), (bv_t, bv),
                         (ident_t, ident)]:
                nc.sync.dma_start(t[:], d)
            for t, d in [(wvd_t, wvd), (wvl1_t, wvl1), (wvl2_t, wvl2),
                         (wvs_t, wvs)]:
                nc.gpsimd.dma_start(t[:], d)

            v_heads = [vall_pool.tile([128, DV * NPOS], F16, name=f"vh{h}")
                       for h in range(NH)]

            import contextlib
            rep_ctx = (tc.For_i(0, reps, 1) if reps > 1
                       else contextlib.nullcontext())
            with rep_ctx:
                logit_ps = [plogit.tile([128, 128], F32, tag="logit",
                                        name=f"logit{h}") for h in range(NH)]

                def conv_qk_tile(qp, kp, x1, x2, m1, m2, y0):
                    """q and k conv for one 16-row output tile: 3 L1-pairs +
                    1 L2-pair + 1 K=64 single each."""
                    x1v = x1[:].rearrange("p (h w) -> p h w", h=HP)
                    x2v = x2[:].rearrange("p (h w) -> p h w", h=HP)
                    x2lo = x2[0:64].rearrange("p (h w) -> p h w", h=HP)
                    m1v = m1[:].rearrange("p (h w) -> p h w", h=HP)
                    m2v = m2[:].rearrange("p (h w) -> p h w", h=HP)
                    m2hi = m2[64:128].rearrange("p (h w) -> p h w", h=HP)
                    for dy in range(3):
                        nc.tensor.matmul(qp[:], wqp_t[:, dy],
                                         x1v[:, y0 + dy:y0 + dy + 16, 0:32],
                                         start=(dy == 0), stop=False)
                    nc.tensor.matmul(qp[:], wqk2_t[:, 0],
                                     x2v[:, y0:y0 + 16, 0:32],
                                     start=False, stop=False)
                    nc.tensor.matmul(qp[:], wqks_t[0:64, 0],
                                     x2lo[:, y0 + 2:y0 + 18, 0:32],
                                     start=False, stop=True)
                    for dy in range(3):
                        nc.tensor.matmul(kp[:], wkp_t[:, dy],
                                         m1v[:, y0 + dy:y0 + dy + 16, 0:32],
                                         start=(dy == 0), stop=False)
                    nc.tensor.matmul(kp[:], wqk2_t[:, 1],
                                     m2v[:, y0:y0 + 16, 0:32],
                                     start=False, stop=False)
                    # single on the upper (row-shifted) half: window rows +1
                    nc.tensor.matmul(kp[:], wqks_t[64:128, 1],
                                     m2hi[:, y0 + 1:y0 + 17, 0:32],
                                     start=False, stop=True)

                def conv_v_tile(vp, td, s1, s2, par, y0):
                    """v conv for one 16-row output tile: 9 D-pair matmuls +
                    3 L1-pairs + 1 L2-pair + 1 single on the solo slice.
                    td is an AP ([128, SLICE] view of a packed D tile)."""
                    tdv = td.rearrange("p (h w) -> p h w", h=HP)
                    i = 0
                    for dy in range(3):
                        for dx in range(3):
                            nc.tensor.matmul(vp[:], wvd_t[:, par, i],
                                             tdv[:, y0 + dy:y0 + dy + 16,
                                                 dx:dx + 32],
                                             start=(i == 0), stop=False)
                            i += 1
                    s1v = s1[:].rearrange("p (h w) -> p h w", h=HP)
                    s2v = s2[:].rearrange("p (h w) -> p h w", h=HP)
                    s2lo = s2[0:64].rearrange("p (h w) -> p h w", h=HP)
                    for dy in range(3):
                        nc.tensor.matmul(vp[:], wvl1_t[:, par, dy],
                                         s1v[:, y0 + dy:y0 + dy + 16, 0:32],
                                         start=False, stop=False)
                    nc.tensor.matmul(vp[:], wvl2_t[:, par],
                                     s2v[:, y0:y0 + 16, 0:32],
                                     start=False, stop=False)
                    nc.tensor.matmul(vp[:], wvs_t[0:64, par],
                                     s2lo[:, y0 + 2:y0 + 18, 0:32],
                                     start=False, stop=True)

                def emit_logits(lslice, qkT_t):
                    hd = lslice // DQ
                    first = (lslice % DQ) == 0
                    last = (lslice % DQ) == DQ - 1
                    for j in range(8):
                        kb = qkT_t[:, KOFF + j * 128:KOFF + (j + 1) * 128]
                        nc.tensor.matmul(
                            logit_ps[hd][:],
                            qkT_t[:, j * 128:(j + 1) * 128], kb,
                            start=(first and j == 0),
                            stop=(last and j == 7),
                            skip_group_check=True)

                def softmax_head(h):
                    negmax = sm_pool.tile([128, 1], F32, tag="negmax",
                                          name="negmax")
                    nc.vector.tensor_reduce(negmax[:], logit_ps[h][:],
                                            op=mybir.AluOpType.max,
                                            axis=mybir.AxisListType.X,
                                            negate=True)
                    attn_exp = sm_pool.tile([128, 128], F32, tag="attn_exp",
                                            name="attn_exp")
                    rowsum = sm_pool.tile([128, 1], F32, tag="rowsum",
                                          name="rowsum")
                    nc.scalar.activation(attn_exp[:], logit_ps[h][:],
                                         mybir.ActivationFunctionType.Exp,
                                         bias=negmax[:], scale=1.0,
                                         accum_out=rowsum[:])
                    recip = sm_pool.tile([128, 1], F32, tag="recip",
                                         name="recip")
                    nc.vector.reciprocal(recip[:], rowsum[:])
                    attn16 = sm_pool.tile([128, 128], F16, tag=f"attn16_{h}",
                                          name=f"attn16_{h}")
                    nc.vector.tensor_scalar_mul(attn16[:], attn_exp[:],
                                                recip[:])
                    return attn16

                def transpose_attn(attn16):
                    # PE-side transpose (identity matmul): any DMA-based
                    # transpose inherits the per-lane DMA FIFO and lands a
                    # full slice late; on the PE it costs ~60ns exactly where
                    # the result is needed.
                    pt = pattn.tile([128, 128], F16, tag="ptT", name="ptT")
                    nc.tensor.matmul(pt[:], attn16[:], ident_t[:],
                                     is_transpose=True)
                    attnT = sm_pool.tile([128, 128], F16, tag="attnT",
                                         name="attnT")
                    nc.vector.tensor_copy(attnT[:], pt[:])
                    return attnT

                def attn_matmuls(h, attnT, c0=0, c1=16, use_act=False):
                    # psum->sbuf copies: DVE-only mid-kernel (the ACT queue
                    # carries the qkT transposes and must not head-of-line
                    # block on them), alternating DVE/ACT at the tail where
                    # the ACT queue is empty; gpsimd (SWDGE) stores in
                    # half-batches so the drain overlaps the copies.
                    ob = None
                    for c in range(c0, c1):
                        po = pconv.tile([128, 512], F32, tag="conv", name="po")
                        nc.tensor.matmul(po[:], attnT[:],
                                         v_heads[h][:, c * 512:(c + 1) * 512],
                                         start=True, stop=True)
                        if c % 4 == 0:
                            ob = ost_pool.tile([128, 2048], F16, tag="ost",
                                               name="ob")
                        dst = ob[:, (c % 4) * 512:(c % 4 + 1) * 512]
                        if use_act and c % 2 == 1:
                            nc.scalar.copy(dst, po[:])
                        else:
                            nc.vector.tensor_copy(dst, po[:])
                        if c % 2 == 1:
                            off = h * DV * NPOS + (c - 1) * 512
                            nc.gpsimd.dma_start(
                                out[:, off:off + 1024],
                                ob[:, (c % 4 - 1) * 512:(c % 4 + 1) * 512])

                def v_slice_tiles(l):
                    """returns (td, s1, s2, par) for output slice lv = l-2."""
                    lv = l - 2
                    par = lv & 1
                    if par == 0:
                        td, ss = d_tiles[lv // 2], lv + 2
                    else:
                        td, ss = d_tiles[(lv + 1) // 2], lv
                    return td, mem1[ss], mem2[ss], par

                mem1, mem2, d_tiles = {}, {}, {}
                pending = []   # deferred logit slices (depth DEFER)
                tp_next = []   # qkT transposes deferred to the next
                               # iteration's sync queue (the ACT queue must
                               # stay clear so the softmax Exp never queues
                               # behind a lane-paced transpose)
                sm0 = None
                attnT0 = None
                for l in range(L):
                    # ---- input loads ----
                    x1 = xin1_pool.tile([128, SLICE], QK_DT, tag="x1",
                                        name="x1")
                    x2 = xin2_pool.tile([128, SLICE], L2_DT, tag="x2",
                                        name="x2")
                    m1 = xm1_pool.tile([128, SLICE], QK_DT, tag="m1",
                                       name="m1")
                    m2 = xm2_pool.tile([128, SLICE], L2_DT, tag="m2",
                                       name="m2")
                    if l < 2:
                        # two column halves: tile-0 matmuls only need < CSPL
                        for t, src in ((x1, xa1), (x2, xa2),
                                       (m1, ma1), (m2, ma2)):
                            nc.sync.dma_start(t[:, 0:CSPL], src[:, l, 0:CSPL])
                        for t, src in ((x1, xa1), (x2, xa2),
                                       (m1, ma1), (m2, ma2)):
                            nc.sync.dma_start(t[:, CSPL:SLICE],
                                              src[:, l, CSPL:SLICE])
                    else:
                        nc.sync.dma_start(x1[:], xa1[:, l])
                        nc.sync.dma_start(x2[:], xa2[:, l])
                        nc.sync.dma_start(m1[:], ma1[:, l])
                        nc.sync.dma_start(m2[:], ma2[:, l])
                    mem1[l], mem2[l] = m1, m2
                    # D-tile prefetch: single slices, spread across iterations
                    dj = D_SCHED.get(l)
                    if dj is not None:
                        dt = xd_pool.tile([128, SLICE], VD_DT, tag="dt",
                                          name="dt")
                        nc.sync.dma_start(dt[:], md[:, dj])
                        d_tiles[dj] = dt[:]

                    # ---- deferred logits (depth DEFER) + head-0 epilogue.
                    # Issued BEFORE the qk conv section so the softmax Exp /
                    # attn transpose / psum copies enqueue on the serial ACT
                    # queue AHEAD of this slice's qkT transpose (which would
                    # otherwise head-of-line-block them by a full slice). ----
                    if len(pending) == DEFER:
                        emit_logits(*pending.pop(0))
                    if l == 8 + DEFER:
                        sm0 = softmax_head(0)
                    elif l == 9 + DEFER:
                        attn_matmuls(0, attnT0, 0, 8, use_act=True)
                    elif l == 10 + DEFER:
                        attn_matmuls(0, attnT0, 8, 16, use_act=True)

                    # ---- previous slice's qkT transpose, one iteration
                    # deferred and AFTER the epilogue block: its eviction is
                    # long done, and the softmax Exp / attn copies enqueue on
                    # the ACT queue ahead of it instead of behind it ----
                    for qkT_p, qc_kc_p in tp_next:
                        nc.scalar.dma_start_transpose(
                            qkT_p[:].rearrange("p (j c) -> p j c",
                                               j=QW // 128),
                            qc_kc_p[:])
                    tp_next = []

                    # ---- q/k convs + eviction into one fused fp16 tile ----
                    qc_kc = qkc_pool.tile([128, QW], F16, tag="qkc",
                                          name="qc_kc")
                    for t in range(2):
                        qp = pconv.tile([128, 512], F32, tag="conv", name="qp")
                        kp = pconv.tile([128, 512], F32, tag="conv", name="kp")
                        conv_qk_tile(qp, kp, x1, x2, m1, m2, t * 16)
                        nc.vector.tensor_scalar_add(
                            qc_kc[:, t * 512:(t + 1) * 512], qp[:], bq_t[:])
                        nc.vector.tensor_scalar_add(
                            qc_kc[:, KOFF + t * 512:KOFF + (t + 1) * 512],
                            kp[:], bk_t[:])

                    # ---- one blocked XBAR transpose per slice; issued next
                    # iteration on sync, except the last slice (no next
                    # iteration — the ACT queue is empty by then) ----
                    qkT_t = qkT_pool.tile([128, QW], F16, tag="qkT",
                                          name="qkT")
                    last = (l == L - 1)
                    if last:
                        nc.scalar.dma_start_transpose(
                            qkT_t[:].rearrange("p (j c) -> p j c",
                                               j=QW // 128),
                            qc_kc[:])
                    else:
                        tp_next.append((qkT_t, qc_kc))
                    pending.append((l, qkT_t))

                    # ---- v conv for output slice l-2 ----
                    if l >= 2:
                        lv = l - 2
                        td, s1, s2, par = v_slice_tiles(l)
                        vh, vd = lv // DV, lv % DV
                        if last:
                            # slices 15/16 logits are ready now; only slice
                            # 17's own logits then wait on its transpose
                            while len(pending) > 1:
                                emit_logits(*pending.pop(0))
                        vp0 = pconv.tile([128, 512], F32, tag="conv",
                                         name="vp0")
                        conv_v_tile(vp0, td, s1, s2, par, 0)
                        nc.vector.tensor_scalar_add(
                            v_heads[vh][:, vd * NPOS:vd * NPOS + 512],
                            vp0[:], bv_t[:])
                        if last:
                            # flush the final logit slice between the two v
                            # tiles: the head-1 softmax then hides behind vp1
                            for p in pending:
                                emit_logits(*p)
                            pending = []
                            sm1 = softmax_head(1)
                        vp1 = pconv.tile([128, 512], F32, tag="conv",
                                         name="vp1")
                        conv_v_tile(vp1, td, s1, s2, par, 16)
                        nc.vector.tensor_scalar_add(
                            v_heads[vh][:, vd * NPOS + 512:(vd + 1) * NPOS],
                            vp1[:], bv_t[:])
                        for s in list(mem1):
                            if s <= l - 3:
                                del mem1[s], mem2[s]

                    # PE transpose of head-0 attn at the END of the slice:
                    # by now the softmax chain (issued at the top) is long
                    # done, so the PE never waits on it
                    if l == 8 + DEFER:
                        attnT0 = transpose_attn(sm0)

                # ---- head-1 epilogue ----
                attnT1 = transpose_attn(sm1)
                attn_matmuls(1, attnT1, use_act=True)
    nc.compile()
    return nc


def prep_inputs(input, memory, wq, bq, wk, bk, wv, bv, **_ignored):
    """Host-side marshalling: packed shifted-image layouts + weight packs."""
    input = np.asarray(input, dtype=np.float32)
    memory = np.asarray(memory, dtype=np.float32)
    wq = np.asarray(wq, dtype=np.float32) * DEPTH_SCALE
    bq = np.asarray(bq, dtype=np.float32) * DEPTH_SCALE
    wk = np.asarray(wk, dtype=np.float32)
    bk = np.asarray(bk, dtype=np.float32)
    wv = np.asarray(wv, dtype=np.float32)
    bv = np.asarray(bv, dtype=np.float32)

    d_qk, d_l2, d_vd = _np(QK_DT), _np(L2_DT), _np(VD_DT)

    def padded_flat(x, dt):
        p = np.zeros((B, CIN, L, HP, WP), dt)
        p[:, :, :, 1:H + 1, 1:W + 1] = x.astype(dt)
        return p.reshape(B, CIN, L, SLICE)

    def shifted_pack(f, s_lo, s_hi):
        """[B, 128, L, SLICE]: rows 0:64 shifted s_lo, rows 64:128 s_hi."""
        p = np.zeros((B, 128, L, SLICE), f.dtype)
        if s_lo:
            p[:, 0:64, :, :SLICE - s_lo] = f[:, :, :, s_lo:]
        else:
            p[:, 0:64] = f
        p[:, 64:128, :, :SLICE - s_hi] = f[:, :, :, s_hi:]
        return p

    xa1 = shifted_pack(padded_flat(input, d_qk), 0, 1)
    xa2 = shifted_pack(padded_flat(input, d_l2), 2, 36)
    ma1 = shifted_pack(padded_flat(memory, d_qk), 0, 1)
    ma2 = shifted_pack(padded_flat(memory, d_l2), 2, 36)
    mfd = padded_flat(memory, d_vd)
    md = np.zeros((B, 128, 9, SLICE), d_vd)
    md[:, 0:64] = mfd[:, :, 0::2]
    md[:, 64:128] = mfd[:, :, 1::2]

    def pair(wa, wb, dt):  # [128 K, 128 M]
        return np.concatenate([wa.T, wb.T], axis=0).astype(dt)

    wq_, wk_ = wq[:, :, 0], wk[:, :, 0]
    wqp = np.stack([pair(wq_[:, :, dy, 0], wq_[:, :, dy, 1], d_qk)
                    for dy in range(3)], axis=1)
    wkp = np.stack([pair(wk_[:, :, dy, 0], wk_[:, :, dy, 1], d_qk)
                    for dy in range(3)], axis=1)
    wqk2 = np.stack([pair(wq_[:, :, 0, 2], wq_[:, :, 1, 2], d_l2),
                     pair(wk_[:, :, 0, 2], wk_[:, :, 1, 2], d_l2)], axis=1)
    wqks = np.stack([pair(wq_[:, :, 2, 2], wq_[:, :, 2, 2], d_l2),
                     pair(wk_[:, :, 2, 2], wk_[:, :, 2, 2], d_l2)], axis=1)

    # v D-pairs: par0 = (dl0|dl1), par1 = (dl1|dl2); tap index i = dy*3+dx
    wvd = np.zeros((128, 2, 9, 128), d_vd)
    for par, (dla, dlb) in [(0, (0, 1)), (1, (1, 2))]:
        for i, (dy, dx) in enumerate((dy, dx) for dy in range(3)
                                     for dx in range(3)):
            wvd[:, par, i] = pair(wv[:, :, dla, dy, dx],
                                  wv[:, :, dlb, dy, dx], d_vd)
    # v solo packs: par0 -> dl2, par1 -> dl0
    wvl1 = np.zeros((128, 2, 3, 128), d_qk)
    wvl2 = np.zeros((128, 2, 128), d_l2)
    wvs = np.zeros((128, 2, 128), d_l2)
    for par, dl in [(0, 2), (1, 0)]:
        for dy in range(3):
            wvl1[:, par, dy] = pair(wv[:, :, dl, dy, 0],
                                    wv[:, :, dl, dy, 1], d_qk)
        wvl2[:, par] = pair(wv[:, :, dl, 0, 2], wv[:, :, dl, 1, 2], d_l2)
        wvs[:, par] = pair(wv[:, :, dl, 2, 2], wv[:, :, dl, 2, 2], d_l2)

    shared = {
        "wqp": wqp, "wkp": wkp, "wqk2": wqk2, "wqks": wqks,
        "wvd": wvd, "wvl1": wvl1, "wvl2": wvl2, "wvs": wvs,
        "bq": bq.reshape(128, 1), "bk": bk.reshape(128, 1),
        "bv": bv.reshape(128, 1),
        "ident": np.eye(128, dtype=np.float16),
    }
    return [{"xa1": np.ascontiguousarray(xa1[b]),
             "xa2": np.ascontiguousarray(xa2[b]),
             "ma1": np.ascontiguousarray(ma1[b]),
             "ma2": np.ascontiguousarray(ma2[b]),
             "md": np.ascontiguousarray(md[b]), **shared} for b in range(B)]


def kernel(**inputs):
    if "nc" not in _CACHE:
        _CACHE["nc"] = build_module()
    nc = _CACHE["nc"]
    in_maps = prep_inputs(**inputs)
    res = bass_utils.run_bass_kernel_spmd(nc, in_maps, core_ids=list(range(B)))
    out = np.stack([res.results[b]["out"].reshape(COUT, LV, H, W)
                    for b in range(B)])
    return out.astype(np.float32)
